# revision 12
# baseline (speedup 1.0000x reference)
"""Trainium2 Bass kernel for nn_ApproxExp_FXP32in16out14 (histogram_binning).

Reference semantics: fixed-point piecewise-linear LUT approximation of exp(x)
over 17 uniform breakpoints on [-10, 4] (FXP32.16 in, FXP16.14 out), including
int32-wraparound artifacts of the torch reference in segments 14/15.

The reference map factors exactly (up to the per-segment LUT rounding of
~0.35% max) as

    out(x) = rho * exp(0.875*k - 10) * ((z - k) + 0.5 + 1/rho + 1/32768)

with z = (8/7)x + 153/14, k = rne(z), rho = e^0.875 - 1.  The host ships
u = fp16(z + 0.5 + 1/rho + 1/32768) -- ALL affine constants folded into one
fused scale+cast pass (halving input HBM traffic vs fp32) -- so the device
needs only three DVE ops plus the exp, all in their fast 16-bit modes:

  DVE     : Kd kq[:, KS:] = i16(u - C0)              (TS 4x, rne == k)
            V  v  = u - kq                             (TT 2x)  -> fp16
            O  out = v * ys                            (TT 2x)  -> fp16
  ScalarE : Ks kq[:, :KS] = i16(u - C0)               (Copy act)
            E  ys = Exp(0.875*kq + (ln rho - 10))               -> fp16

with C0 = 0.5 + 1/rho + 1/32768.  The K column split (KS) balances DVE
(~5.3us/tile) and ScalarE (~5.8us/tile) just under the ~5.9us/tile DMA
cadence; the last tile's V/E/O + output DMA run as four quarter-width jobs
to shrink the pipeline drain tail.

v is slaved to kq (v = u - kq), so any rounding direction of the K op at
segment-boundary ties yields a consistent (k, v) pair: the model is exactly
continuous across breakpoints ((1+1/rho)/(e^0.875/rho) == 1), making ties
benign.  Output fp16 (~0.2% rel rms total; the gate is 2e-2), upcast on
host.  A deterministic ~0.3% of elements (int32-wraparound bands at
x>=2.7773, the x>=4 clamp, deep tail x<-4.7) is recomputed exactly on host.

Sharding: pure data parallel, leading dim 64 -> 8 cores x 8.
"""

import math
from contextlib import ExitStack

import numpy as np

import concourse.bass as bass
import concourse.mybir as mybir
from concourse.bass_utils import run_bass_kernel_spmd

# ---------------------------------------------------------------- constants
FULL_SHAPE = (64, 4096, 1024)
N_CORES = 8
TILES, P, F = 64, 128, 4096  # per-core: 64 tiles of [128, 4096]
NBUF = 4
KS = 1152                    # quantize cols [0:KS] on ScalarE, [KS:] on DVE
NQ = 4                       # last tile's V/E/O/out-DMA split into NQ quarters

RHO = math.exp(0.875) - 1.0
V_ADD = 0.5 + 1.0 / RHO + 1.0 / 32768.0  # folded into u on host
U_BIAS = 153.0 / 14.0 + V_ADD        # host: u = (8/7)x + U_BIAS
K_ADD = -V_ADD                       # kq = rne(u + K_ADD) == rne(z)
E_SCALE = 0.875
E_BIAS = math.log(RHO) - 10.0        # ys = exp(E_SCALE*k + E_BIAS)

# host-fixup region boundaries (float32 compares on raw x)
FIX_HI = np.float32(2.7773)          # below first int32-wrap threshold (2.77735)
FIX_LO = np.float32(-4.7)            # deep tail: LUT quantization breaks the model

# ------------------------------------------------------------ bass builder
_NC = None


def _build_nc(tiles: int = TILES) -> bass.Bass:
    f32, i16, fp16 = mybir.dt.float32, mybir.dt.int16, mybir.dt.float16
    A = mybir.AluOpType
    nc = bass.Bass()
    u_ext = nc.declare_dram_parameter("u", [tiles, P, F], fp16, isOutput=False)
    o_ext = nc.declare_dram_parameter("out", [tiles, P, F], fp16, isOutput=True)

    # [128,1] constant for the Exp activation bias (const_aps only has 0/1).
    bias_t = nc.alloc_sbuf_tensor("const-ebias", [P, 1], f32)
    nc.gpsimd.memset(bias_t.ap(), E_BIAS)
    nc.all_engine_barrier()
    e_bias_ap = bias_t.ap()

    ctx = ExitStack()
    ut = [ctx.enter_context(nc.sbuf_tensor(f"ut{j}", [P, F], fp16)) for j in range(NBUF)]
    kq = [ctx.enter_context(nc.sbuf_tensor(f"kq{j}", [P, F], i16)) for j in range(NBUF)]
    vh = [ctx.enter_context(nc.sbuf_tensor(f"vh{j}", [P, F], fp16)) for j in range(NBUF)]
    ys = [ctx.enter_context(nc.sbuf_tensor(f"ys{j}", [P, F], fp16)) for j in range(NBUF)]
    ot = [ctx.enter_context(nc.sbuf_tensor(f"ot{j}", [P, F], fp16)) for j in range(NBUF)]
    # per-buffer-slot DMA semaphores: at most one in-flight DMA per sem, so a
    # waiter on >=16*n can't be satisfied by interleaved partial completions
    # of two DMAs (the 16 per-engine increments of concurrent DMAs interleave).
    s_in = [ctx.enter_context(nc.semaphore(f"s_in{j}")) for j in range(NBUF)]
    s_out = [ctx.enter_context(nc.semaphore(f"s_out{j}")) for j in range(NBUF)]
    s_ks = ctx.enter_context(nc.semaphore("s_ks"))  # ScalarE Ks done (per tile)
    s_kd = ctx.enter_context(nc.semaphore("s_kd"))  # DVE Kd done (per tile)
    s_v = ctx.enter_context(nc.semaphore("s_v"))    # DVE V done (per tile)
    s_y = ctx.enter_context(nc.semaphore("s_y"))    # ScalarE E done (per job)
    s_o = ctx.enter_context(nc.semaphore("s_o"))    # DVE O done (per job)
    block = ctx.enter_context(nc.Block())

    LOOK = NBUF - 1  # input prefetch distance
    last = tiles - 1
    QW = F // NQ  # quarter width for the last tile's drain jobs
    # E/O job counts: tiles 0..last-1 contribute 1 each, the last tile NQ.

    @block.sync
    def _(sync):
        for i in range(min(LOOK, tiles)):
            sync.dma_start(out=ut[i % NBUF][:], in_=u_ext[i]).then_inc(
                s_in[i % NBUF], 16
            )
        for i in range(tiles - LOOK):
            # ut[(i+LOOK)%NBUF] is read by Ks/Kd/V of tile i-1 only
            if i >= 1:
                sync.wait_ge(s_ks, i)  # Ks(i-1) done
                sync.wait_ge(s_v, i)   # V(i-1) done => Kd(i-1) done too
            sync.dma_start(
                out=ut[(i + LOOK) % NBUF][:], in_=u_ext[i + LOOK]
            ).then_inc(s_in[(i + LOOK) % NBUF], 16)

    @block.scalar
    def _(scalar):
        def stage_ks(i):
            """Ks(i): quantize cols [0:KS] on the activation engine."""
            j = i % NBUF
            scalar.wait_ge(s_in[j], 16 * (i // NBUF + 1))
            if i >= NBUF:
                scalar.wait_ge(s_y, i - NBUF + 1)  # kq slot: E(i-NBUF) done
                scalar.wait_ge(s_v, i - NBUF + 1)  # kq slot: V(i-NBUF) done
            nc.scalar.activation(
                kq[j][:, :KS], ut[j][:, :KS],
                mybir.ActivationFunctionType.Copy,
                bias=K_ADD, scale=1.0,
            ).then_inc(s_ks, 1)

        stage_ks(0)
        for i in range(tiles):
            j = i % NBUF
            # issue next tile's Ks before E(i): DVE's V(i+1) waits on it
            if i + 1 < last:
                stage_ks(i + 1)
            if i < last:
                scalar.wait_ge(s_ks, i + 1)  # own Ks(i) retired (race-det sync)
            scalar.wait_ge(s_kd, i + 1)  # DVE Kd(i) done
            if i >= NBUF:
                scalar.wait_ge(s_o, i - NBUF + 1)  # ys slot free (O(i-NBUF))
            if i < last:
                nc.scalar.activation(
                    ys[j][:], kq[j][:], mybir.ActivationFunctionType.Exp,
                    bias=e_bias_ap, scale=E_SCALE,
                ).then_inc(s_y, 1)
                # out-DMA of the previous tile on the ACT HWDGE queue: O(i-1)
                # is all but guaranteed done by the end of E(i), so this
                # rarely stalls and the sync queue free-runs on inputs.
                if i >= 1:
                    scalar.wait_ge(s_o, i)
                    scalar.dma_start(
                        out=o_ext[i - 1], in_=ot[(i - 1) % NBUF][:]
                    ).then_inc(s_out[(i - 1) % NBUF], 16)
            else:
                # drain: E in NQ quarter jobs so O/out-DMA can chase each one
                for q in range(NQ):
                    lo, hi = q * QW, (q + 1) * QW
                    nc.scalar.activation(
                        ys[j][:, lo:hi], kq[j][:, lo:hi],
                        mybir.ActivationFunctionType.Exp,
                        bias=e_bias_ap, scale=E_SCALE,
                    ).then_inc(s_y, 1)
                    if q == 0:
                        scalar.wait_ge(s_o, last)  # O(last-1) done
                        scalar.dma_start(
                            out=o_ext[last - 1], in_=ot[(last - 1) % NBUF][:]
                        ).then_inc(s_out[(last - 1) % NBUF], 16)
                for q in range(NQ):
                    lo, hi = q * QW, (q + 1) * QW
                    scalar.wait_ge(s_o, last + q + 1)  # O quarter q done
                    scalar.dma_start(
                        out=o_ext[last][:, lo:hi], in_=ot[j][:, lo:hi]
                    ).then_inc(s_out[j], 16)

    @block.vector
    def _(vector):
        def stage_kv(i):
            """Kd(i) + V(i): the tile's DVE quantize + fraction ops."""
            j = i % NBUF
            vector.wait_ge(s_in[j], 16 * (i // NBUF + 1))
            if i >= NBUF:
                vector.wait_ge(s_y, i - NBUF + 1)  # kq slot free (E(i-NBUF))
            # the last tile runs Kd over the full width (no Ks on ScalarE,
            # which shortens the serial drain chain there)
            kslice = slice(KS, None) if i < last else slice(None)
            nc.vector.tensor_scalar_add(
                out=kq[j][:, kslice], in0=ut[j][:, kslice], scalar1=K_ADD,
            ).then_inc(s_kd, 1)
            if i >= NBUF:
                vector.wait_ge(s_o, i - NBUF + 1)  # vh slot free (O(i-NBUF))
            vector.wait_ge(s_kd, i + 1)  # own Kd(i) retired (race-det sync)
            if i < last:
                vector.wait_ge(s_ks, i + 1)  # ScalarE Ks(i) done (kq[:, :KS])
            nc.vector.tensor_tensor(
                out=vh[j][:], in0=ut[j][:], in1=kq[j][:], op=A.subtract,
            ).then_inc(s_v, 1)

        stage_kv(0)
        for i in range(tiles):
            j = i % NBUF
            # issue next tile's Kd/V before O(i): O waits on E(i), and E(i+1)
            # needs Kd(i+1) -- this keeps that off the cross-engine cycle.
            if i + 1 < tiles:
                stage_kv(i + 1)
            vector.wait_ge(s_v, i + 1)  # own V(i) retired (race-detector sync)
            if i >= NBUF:
                vector.wait_ge(s_out[j], 16 * (i // NBUF))  # ot slot free
            if i < last:
                vector.wait_ge(s_y, i + 1)  # E(i) done
                nc.vector.tensor_tensor(
                    out=ot[j][:], in0=vh[j][:], in1=ys[j][:], op=A.mult,
                ).then_inc(s_o, 1)
            else:
                for q in range(NQ):
                    lo, hi = q * QW, (q + 1) * QW
                    vector.wait_ge(s_y, last + q + 1)  # E quarter q done
                    nc.vector.tensor_tensor(
                        out=ot[j][:, lo:hi], in0=vh[j][:, lo:hi],
                        in1=ys[j][:, lo:hi], op=A.mult,
                    ).then_inc(s_o, 1)

    ctx.close()
    return nc


def _get_nc() -> bass.Bass:
    global _NC
    if _NC is None:
        _NC = _build_nc()
    return _NC


# ------------------------------------------------- exact host-side reference
_XP = np.round(np.linspace(-10.0, 4.0, 17) * 65536.0).astype(np.int64)
_YV = np.round(np.exp(np.linspace(-10.0, 4.0, 17)) * 16384.0).astype(np.int64)
_DY = np.diff(_YV)


def _reference_exact(xs: np.ndarray) -> np.ndarray:
    """Bit-faithful int32 reference for a (small) subset of elements."""
    x_int = np.rint(xs.astype(np.float64) * 65536.0).astype(np.int64)
    mask_low = x_int <= _XP[0]
    mask_high = x_int >= _XP[-1]
    xc = np.clip(x_int, _XP[0], _XP[-1])
    idx = np.clip(np.searchsorted(_XP, xc, side="left") - 1, 0, 15)
    dxv = xc - _XP[idx]
    t_fx = ((dxv << 14) + 28672) // 57344
    prod = t_fx * _DY[idx] + 8192
    pm = prod & 0xFFFFFFFF
    S = np.where(pm >= 1 << 31, pm - (1 << 32), pm)
    interp = _YV[idx] + (S >> 14)
    out_int = np.where(mask_low, _YV[0], np.where(mask_high, _YV[-1], interp))
    return (out_int.astype(np.float32) / np.float32(16384.0)).astype(np.float32)


def _host_fixup(x_flat: np.ndarray, out_flat: np.ndarray) -> None:
    sel = (x_flat >= FIX_HI) | (x_flat < FIX_LO)
    idxs = np.flatnonzero(sel)
    if idxs.size:
        out_flat[idxs] = _reference_exact(x_flat[idxs])


_last_results = None


def kernel(x: np.ndarray) -> np.ndarray:
    assert x.shape == FULL_SHAPE and x.dtype == np.float32, (x.shape, x.dtype)
    nc = _get_nc()
    per = FULL_SHAPE[0] // N_CORES
    u16 = (x * np.float32(8.0 / 7.0) + np.float32(U_BIAS)).astype(np.float16)
    in_maps = [
        {"u": np.ascontiguousarray(u16[i * per : (i + 1) * per]).reshape(TILES, P, F)}
        for i in range(N_CORES)
    ]
    global _last_results
    res = run_bass_kernel_spmd(nc, in_maps, core_ids=list(range(N_CORES)))
    _last_results = res
    out = np.concatenate(
        [
            r["out"].astype(np.float32).reshape(per, FULL_SHAPE[1], FULL_SHAPE[2])
            for r in res.results
        ],
        axis=0,
    )
    _host_fixup(x.ravel(), out.ravel())
    return out
